# revision 29
# baseline (speedup 1.0000x reference)
"""Trainium2 Bass kernel for multi-head attention (B=2, S=2048, D=1024, H=16).

Sharding: 8 cores = 2 batches x 4 head-groups. Core c handles batch c//4 and
heads [4*(c%4), 4*(c%4)+4). Each core computes its 4 heads' Q/K/V projections
(column-sharded), attention, and a partial output projection over its 256
rows of Wo. Host sums the 4 partials per batch (tensor-parallel unshard).

Device-side layout choices:
  - Q/K kept transposed per head: QhT/KhT [hd, s] so logits are computed
    directly in [k, q] (transposed) orientation; attn@V consumes exp
    weights with k on partitions, producing attnT [hd, q] which feeds the
    output projection as the stationary operand without any transposes.
  - Softmax denominators ride in the attn@V matmuls: each head's V
    stationary carries a 65th column of ones, so PSUM row 64 of the av
    tile accumulates sum_k(exp) for free. This removes the separate
    ones-matmul denominator stream (25% of the attention-stream PE work).
    Normalization: DVE reciprocal of row 64 (bf16), then a K=1 ones-matmul
    broadcasts 1/den across the 64 head partitions (512-cycle stream per
    head per q-tile), and DVE multiplies av by it into attnT.
  - exp weights live in per-k-tile rotating buffers (bufs=3) so ACT's
    exp of tile c overlaps PE's attn@V of tile c-1. The ACT-engine
    softmax exp (S^2*H/8 = 16.8M elems/core at 1 elem/lane/cycle,
    ~109us) is the engine roofline for this problem on 8 cores.
  - Software-pipelined schedule: weights are loaded once (resident
    across loop iterations); the body starts attention immediately on
    Q0/K0/V0 projected by the pre-loop prologue (or the previous
    iteration's tail), and the remaining projection chunks are woven
    just-in-time into the ACT-bound attention k-tile loops. Each
    q-tile's normalize/output-projection is spread over the next
    q-tile's first k-tiles so PE never drains while DVE runs the
    reciprocal chain. Input-stream DMAs are issued several k-tiles
    ahead of their consuming projections.
  - All matmuls in bf16 (fp32 PSUM accumulation); softmax in fp32.
"""

import os
import sys

import numpy as np

sys.path.insert(0, "/opt/trn_rl_repo")

import ml_dtypes

B, S, D, H = 2, 2048, 1024, 16
HD = D // H          # 64 head dim
NCORES = 8
CPB = 4              # cores per batch
NHC = 4              # heads per core
COLS = NHC * HD      # 256 projection columns per core
VW = HD + 1          # V stationary width per head (64 V cols + ones col)
NG = 2               # groups of 128 cols (head pairs)
QTS = 512            # q tile size
NQT = S // QTS       # 4
KTS = 128            # k tile size
NKT = S // KTS       # 16
NDC = D // 128       # 8 contraction chunks for projections
DOT = 512            # out-proj column tile
NDO = D // DOT       # 2
SCALE = 1.0 / float(np.sqrt(HD))

_PROGRAMS = {}


def _build_program(loopn=1, unroll=1):
    import concourse.bass as bass
    import concourse.tile as tile
    from concourse import bacc
    import concourse.mybir as mybir

    f32 = mybir.dt.float32
    bf16 = mybir.dt.bfloat16
    AF = mybir.ActivationFunctionType
    PSUM = bass.MemorySpace.PSUM

    nc = bacc.Bacc("TRN2", target_bir_lowering=False, debug=False)

    qT_d = nc.dram_tensor("qT", [D, S], bf16, kind="ExternalInput")
    kT_d = nc.dram_tensor("kT", [D, S], bf16, kind="ExternalInput")
    vT_d = nc.dram_tensor("vT", [D, S], bf16, kind="ExternalInput")
    wq_d = nc.dram_tensor("wq", [D, COLS], bf16, kind="ExternalInput")
    wk_d = nc.dram_tensor("wk", [D, COLS], bf16, kind="ExternalInput")
    wv_d = nc.dram_tensor("wv", [D, COLS], bf16, kind="ExternalInput")
    wo_d = nc.dram_tensor("wo", [COLS, D], bf16, kind="ExternalInput")
    bqr_d = nc.dram_tensor("bqr", [128, NG], f32, kind="ExternalInput")
    bkr_d = nc.dram_tensor("bkr", [128, NG], f32, kind="ExternalInput")
    bvr_d = nc.dram_tensor("bvr", [128, COLS], f32, kind="ExternalInput")
    bor_d = nc.dram_tensor("bor", [128, D], f32, kind="ExternalInput")
    # output staged in bf16: halves the out-store DMA traffic; the host
    # sums the per-core partials in f32 (costs ~1e-3 rel err)
    out_d = nc.dram_tensor("out", [S, D], bf16, kind="ExternalOutput")

    with tile.TileContext(nc) as tc:
        with (
            tc.tile_pool(name="persist", bufs=1) as persist,
            tc.tile_pool(name="wpool", bufs=1) as wpool,
            tc.tile_pool(name="xstream", bufs=2) as xstream,
            tc.tile_pool(name="rpool", bufs=4) as rpool,
            tc.tile_pool(name="outstage", bufs=3) as outstage,
            tc.tile_pool(name="expp", bufs=3) as expp,
            tc.tile_pool(name="lpp", bufs=2, space=PSUM) as lpp,
            tc.tile_pool(name="avp", bufs=4, space=PSUM) as avp,
        ):
            # ---- persistent SBUF tiles ----
            QhT = persist.tile([128, NG, S], bf16)       # [p, grp, s]
            KhT = persist.tile([128, NG, S], bf16)
            # V per ktile: 4 heads x (64 V cols + 1 ones col)
            Vh = persist.tile([128, NKT, NHC, VW], bf16)
            attnT = persist.tile([128, NG, S], bf16)
            wo_sb = persist.tile([128, NG, D], bf16)
            bqr_sb = persist.tile([128, NG], f32)
            bkr_sb = persist.tile([128, NG], f32)
            bv_bc = persist.tile([128, COLS], f32)
            bo_bc = persist.tile([128, D], f32)
            ones1 = persist.tile([1, HD], bf16)          # bcast stationary

            warm_sb = persist.tile([128, 1], f32)
            nc.vector.memset(ones1[:], 1.0)
            nc.vector.memset(Vh[:, :, :, HD:VW], 1.0)    # ones columns
            nc.vector.memset(warm_sb[:], 0.0)
            # hoist the ACT exp-table load to t=0 (overlaps the input DMAs)
            nc.scalar.activation(warm_sb[:], warm_sb[:],
                                 mybir.ActivationFunctionType.Exp)
            nc.sync.dma_start(out=bqr_sb[:], in_=bqr_d[:])
            nc.sync.dma_start(out=bkr_sb[:], in_=bkr_d[:])
            nc.sync.dma_start(out=bv_bc[:], in_=bvr_d[:])
            nc.sync.dma_start(out=bo_bc[:], in_=bor_d[:])

            wq_sb = wpool.tile([128, NDC, COLS], bf16, tag="wq", name="wq_sb")
            wk_sb = wpool.tile([128, NDC, COLS], bf16, tag="wk", name="wk_sb")
            wv_sb = wpool.tile([128, NDC, COLS], bf16, tag="wv", name="wv_sb")

            qT_r = qT_d[:].rearrange("(c p) (t n) -> p c t n", p=128, n=QTS)
            kT_r = kT_d[:].rearrange("(c p) (t n) -> p c t n", p=128, n=QTS)
            vT_r = vT_d[:].rearrange("(c p) (t n) -> p c t n", p=128, n=QTS)
            bv_v = bv_bc[:].rearrange("p (h w) -> p h w", w=HD)

            # Transient PSUM tiles emitted while the 4 av accumulators
            # are open MUST NOT rotate through the avp pool (PE is
            # in-order; a slot-reuse wait on an open accumulator
            # deadlocks). Pre-loop prologue work (before any av alloc) uses
            # avp; everything woven into the k-tile loops rides lpp slots.
            def prefetch_x(x_r, xtag, qt):
                x_sb = xstream.tile([128, NDC, QTS], bf16, tag=xtag,
                                    name="x_sb")
                nc.sync.dma_start(out=x_sb[:], in_=x_r[:, :, qt, :])
                return x_sb

            def emit_qk_proj(dst, w_sb, x_r, b_sb, xtag, qt, pool, tag,
                             x_sb=None):
                if x_sb is None:
                    x_sb = prefetch_x(x_r, xtag, qt)
                for g in range(NG):
                    ps = pool.tile([128, QTS], f32, tag=tag, name="qk_ps")
                    for dc in range(NDC):
                        nc.tensor.matmul(
                            ps[:],
                            w_sb[:, dc, g * 128:(g + 1) * 128],
                            x_sb[:, dc, :],
                            start=(dc == 0), stop=(dc == NDC - 1),
                        )
                    nc.vector.tensor_scalar_add(
                        dst[:, g, qt * QTS:(qt + 1) * QTS],
                        ps[:], b_sb[:, g:g + 1])

            def emit_v_proj(vt, pool, tag, v_sb=None):
                # projects V rows [vt*512, (vt+1)*512) = ktiles 4vt..4vt+3
                if v_sb is None:
                    v_sb = prefetch_x(vT_r, "vx", vt)
                for sst in range(QTS // 128):
                    st = vt * 4 + sst
                    v_ps = pool.tile([128, COLS], f32, tag=tag, name="v_ps")
                    for dc in range(NDC):
                        nc.tensor.matmul(
                            v_ps[:],
                            v_sb[:, dc, sst * 128:(sst + 1) * 128],
                            wv_sb[:, dc, :],
                            start=(dc == 0), stop=(dc == NDC - 1),
                        )
                    nc.vector.tensor_add(
                        Vh[:, st, :, 0:HD],
                        v_ps[:].rearrange("p (h w) -> p h w", w=HD),
                        bv_v)

            def emit_av(c, expc, av_t):
                for h in range(NHC):
                    nc.tensor.matmul(
                        av_t[h][:, :],
                        Vh[:, c, h, :],
                        expc[:, h, :],
                        start=(c == 0), stop=(c == NKT - 1),
                        tile_position=(0, 0),
                        skip_group_check=True,
                    )

            def emit_normalize(qt, av_t):
                # attnT = av[0:64] / av[64] (ones-column denominator row):
                # reciprocal -> K=1 ones-matmul broadcast across the 64
                # head partitions -> multiply.
                q0 = qt * QTS
                rcp_sb = rpool.tile([1, NHC, QTS], bf16, tag="rcp",
                                    name="rcp_sb")
                with nc.allow_low_precision(
                        reason="bf16 reciprocal of softmax denominator"):
                    for h in range(NHC):
                        nc.vector.reciprocal(rcp_sb[0:1, h, :],
                                             av_t[h][HD:VW, :])
                rb = lpp.tile([128, 2, QTS], f32, tag="Lp", name="rb")
                for ph in range(NG):
                    for h2 in range(2):
                        h = 2 * ph + h2
                        nc.tensor.matmul(
                            rb[h2 * 64:h2 * 64 + 64, ph, :],
                            ones1[:, :],
                            rcp_sb[0:1, h, :],
                            start=True, stop=True,
                            tile_position=(0, h2 * 64),
                            skip_group_check=True,
                        )
                # DVE reads at most one PSUM operand: stage rb in SBUF
                rb_sb = rpool.tile([128, 2, QTS], f32, tag="rb",
                                   name="rb_sb")
                for ph in range(NG):
                    nc.vector.tensor_copy(rb_sb[:, ph, :], rb[:, ph, :])
                    for h2 in range(2):
                        h = 2 * ph + h2
                        nc.vector.tensor_mul(
                            attnT[h2 * 64:h2 * 64 + 64, ph, q0:q0 + QTS],
                            av_t[h][0:HD, :],
                            rb_sb[h2 * 64:h2 * 64 + 64, ph, :])

            def emit_outproj(qt, qs_list=None):
                # output projection for q tile qt (partial, 256 Wo rows)
                for qs in (range(QTS // 128) if qs_list is None else qs_list):
                    r0 = qt * QTS + qs * 128
                    for do in range(NDO):
                        op_ps = lpp.tile([128, DOT], f32, tag="Lp",
                                         name="op_ps")
                        for ch in range(NG):
                            nc.tensor.matmul(
                                op_ps[:],
                                attnT[:, ch, r0:r0 + 128],
                                wo_sb[:, ch, do * DOT:(do + 1) * DOT],
                                start=(ch == 0), stop=(ch == NG - 1),
                            )
                        st_t = outstage.tile([128, DOT], bf16, tag="st",
                                             name="st_t")
                        nc.vector.tensor_add(st_t[:], op_ps[:],
                                             bo_bc[:, do * DOT:(do + 1) * DOT])
                        nc.sync.dma_start(
                            out=out_d[r0:r0 + 128, do * DOT:(do + 1) * DOT],
                            in_=st_t[:])

            def emit_recips(av_t):
                rcp_sb = rpool.tile([1, NHC, QTS], bf16, tag="rcp",
                                    name="rcp_sb")
                with nc.allow_low_precision(
                        reason="bf16 reciprocal of softmax denominator"):
                    for h in range(NHC):
                        nc.vector.reciprocal(rcp_sb[0:1, h, :],
                                             av_t[h][HD:VW, :])
                return rcp_sb

            def emit_finish_norm(qt, av_t, rcp_sb):
                q0 = qt * QTS
                rb = lpp.tile([128, 2, QTS], f32, tag="Lp", name="rb")
                for ph in range(NG):
                    for h2 in range(2):
                        h = 2 * ph + h2
                        nc.tensor.matmul(
                            rb[h2 * 64:h2 * 64 + 64, ph, :],
                            ones1[:, :],
                            rcp_sb[0:1, h, :],
                            start=True, stop=True,
                            tile_position=(0, h2 * 64),
                            skip_group_check=True,
                        )
                # DVE reads at most one PSUM operand: stage rb in SBUF
                rb_sb = rpool.tile([128, 2, QTS], f32, tag="rb",
                                   name="rb_sb")
                for ph in range(NG):
                    nc.vector.tensor_copy(rb_sb[:, ph, :], rb[:, ph, :])
                    for h2 in range(2):
                        h = 2 * ph + h2
                        nc.vector.tensor_mul(
                            attnT[h2 * 64:h2 * 64 + 64, ph, q0:q0 + QTS],
                            av_t[h][0:HD, :],
                            rb_sb[h2 * 64:h2 * 64 + 64, ph, :])

            def body(_iv=None):
                # ---- pipelined schedule ----
                # The body starts with Q tile 0 / K chunk 0 / V ktiles 0-3
                # already projected: the pre-loop prologue does it for the
                # first iteration, and each body's tail fills re-project them
                # for the next iteration (the DRAM inputs are constant within
                # a launch, so re-projecting yields identical values).
                #
                # Deferred projection work is woven into the attention k-tile
                # loops (which are ACT-bound at ~2.1us/ktile vs PE ~1.7us):
                # qt0 carries K chunks 1-3 + V tiles 1-3 (due just before
                # their consuming ktiles) and Q1; Q2/Q3/Q0' land at the heads
                # of qt1/qt2/qt3, giving PE independent work while DVE runs
                # the previous q-tile's reciprocals. K0'/V0' interleave with
                # the qt3 tail's normalize steps for the same reason.
                # Fill schedule: x-stream DMAs are issued ("dma")
                # several ktiles before their projection consumes them
                # ("cmp"), so the ~3us DMA never stalls PE. px holds the
                # prefetched tiles.
                px = {}
                QK, VV = "qk", "v"

                def dma_fill(kind, xtag, qt):
                    if kind == QK:
                        px[(xtag, qt)] = prefetch_x(
                            qT_r if xtag == "qx" else kT_r, xtag, qt)
                    else:
                        px[("vx", qt)] = prefetch_x(vT_r, "vx", qt)

                def cmp_fill(kind, xtag, qt):
                    if kind == QK:
                        if xtag == "qx":
                            emit_qk_proj(QhT, wq_sb, qT_r, bqr_sb, "qx", qt,
                                         lpp, "Lp", x_sb=px.pop(("qx", qt)))
                        else:
                            emit_qk_proj(KhT, wk_sb, kT_r, bkr_sb, "kx", qt,
                                         lpp, "Lp", x_sb=px.pop(("kx", qt)))
                    else:
                        emit_v_proj(qt, lpp, "Lp", v_sb=px.pop(("vx", qt)))

                fill = {
                    (0, 0): lambda: dma_fill(QK, "kx", 1),
                    (0, 1): lambda: dma_fill(VV, "vx", 1),
                    (0, 2): lambda: cmp_fill(QK, "kx", 1),
                    (0, 3): lambda: cmp_fill(VV, "vx", 1),
                    (0, 4): lambda: dma_fill(QK, "kx", 2),
                    (0, 5): lambda: dma_fill(VV, "vx", 2),
                    (0, 6): lambda: cmp_fill(QK, "kx", 2),
                    (0, 7): lambda: cmp_fill(VV, "vx", 2),
                    (0, 8): lambda: dma_fill(QK, "kx", 3),
                    (0, 9): lambda: dma_fill(VV, "vx", 3),
                    (0, 10): lambda: cmp_fill(QK, "kx", 3),
                    (0, 11): lambda: cmp_fill(VV, "vx", 3),
                    (0, 12): lambda: dma_fill(QK, "qx", 1),
                    (0, 14): lambda: cmp_fill(QK, "qx", 1),
                    (0, 15): lambda: dma_fill(QK, "qx", 2),
                    (1, 0): lambda: cmp_fill(QK, "qx", 2),
                    (1, 12): lambda: dma_fill(QK, "qx", 3),
                    (2, 0): lambda: cmp_fill(QK, "qx", 3),
                    (2, 12): lambda: dma_fill(QK, "qx", 0),
                    (3, 0): lambda: cmp_fill(QK, "qx", 0),
                    (3, 4): lambda: dma_fill(QK, "kx", 0),
                    (3, 6): lambda: dma_fill(VV, "vx", 0),
                }

                prev = None  # (qt, av_t) awaiting normalize + outproj
                for qt in range(NQT):
                    q0 = qt * QTS
                    av_t = [avp.tile([VW, QTS], f32, tag="av", name=f"av{h}")
                            for h in range(NHC)]
                    expc_tiles = [None] * NKT

                    for c in range(NKT):
                        expc = expp.tile([128, NHC, QTS], bf16, tag="expc",
                                         name="expc")
                        expc_tiles[c] = expc
                        for ph in range(NG):
                            lp = lpp.tile([128, 2, QTS], f32, tag="Lp", name="lp")
                            for h2 in range(2):
                                pb = h2 * 64
                                nc.tensor.matmul(
                                    lp[:, h2, :],
                                    KhT[pb:pb + 64, ph, c * 128:(c + 1) * 128],
                                    QhT[pb:pb + 64, ph, q0:q0 + QTS],
                                    start=True, stop=True,
                                    tile_position=(pb, 0),
                                )
                            nc.scalar.activation(
                                expc[:, 2 * ph:2 * ph + 2, :],
                                lp[:],
                                AF.Exp, scale=SCALE,
                            )
                        # fill first: gives PE independent work to chew while
                        # DVE runs the previous q-tile's reciprocals
                        f = fill.pop((qt, c), None)
                        if f is not None:
                            f()
                        if prev is not None:
                            if c == 0:
                                emit_finish_norm(prev[0], prev[1], prev[2])
                            elif c in (1, 2, 3, 4):
                                emit_outproj(prev[0], [c - 1])
                                if c == 4:
                                    prev = None
                        if c > 0:
                            emit_av(c - 1, expc_tiles[c - 1], av_t)
                    emit_av(NKT - 1, expc_tiles[NKT - 1], av_t)
                    rcp_sb = emit_recips(av_t)
                    prev = (qt, av_t, rcp_sb)

                # tail: qt3's normalize + outproj, interleaved with the next
                # iteration's K0/V0 projections (x prefetched in qt3) so PE
                # has independent work while DVE runs the rcp/multiply chain
                emit_qk_proj(KhT, wk_sb, kT_r, bkr_sb, "kx", 0, lpp, "Lp",
                             x_sb=px.pop(("kx", 0)))
                emit_finish_norm(prev[0], prev[1], prev[2])
                emit_v_proj(0, lpp, "Lp", v_sb=px.pop(("vx", 0)))
                emit_outproj(3, [0, 1])
                emit_outproj(3, [2, 3])

            # ---- pre-loop prologue: weights resident for the whole launch,
            # first iteration's Q0/K0/V0 projections ----
            nc.sync.dma_start(out=wq_sb[:],
                              in_=wq_d[:].rearrange("(c p) n -> p c n", p=128))
            emit_qk_proj(QhT, wq_sb, qT_r, bqr_sb, "qx", 0, avp, "av")
            nc.sync.dma_start(out=wk_sb[:],
                              in_=wk_d[:].rearrange("(c p) n -> p c n", p=128))
            emit_qk_proj(KhT, wk_sb, kT_r, bkr_sb, "kx", 0, avp, "av")
            nc.sync.dma_start(out=wv_sb[:],
                              in_=wv_d[:].rearrange("(c p) n -> p c n", p=128))
            emit_v_proj(0, avp, "av")
            nc.sync.dma_start(out=wo_sb[:],
                              in_=wo_d[:].rearrange("(c p) d -> p c d", p=128))

            if loopn == 1:
                for _ in range(unroll):
                    body()
            else:
                with tc.For_i(0, loopn, 1) as iv:
                    body(iv)

    nc.compile()
    return nc


def _get_program(loopn=1, unroll=1):
    key = (loopn, unroll)
    if key not in _PROGRAMS:
        _PROGRAMS[key] = _build_program(loopn, unroll)
    return _PROGRAMS[key]


def make_in_maps(q, k, v, Wq, Wk, Wv, Wo, bq, bk, bv, bo):
    bf = ml_dtypes.bfloat16
    q = np.asarray(q, np.float32)
    k = np.asarray(k, np.float32)
    v = np.asarray(v, np.float32)
    Wq = np.asarray(Wq, np.float32)
    Wk = np.asarray(Wk, np.float32)
    Wv = np.asarray(Wv, np.float32)
    Wo = np.asarray(Wo, np.float32)
    bq = np.asarray(bq, np.float32)
    bk = np.asarray(bk, np.float32)
    bv = np.asarray(bv, np.float32)
    bo = np.asarray(bo, np.float32)

    qT = [np.ascontiguousarray(q[b].T).astype(bf) for b in range(B)]
    kT = [np.ascontiguousarray(k[b].T).astype(bf) for b in range(B)]
    vT = [np.ascontiguousarray(v[b].T).astype(bf) for b in range(B)]

    in_maps = []
    for c in range(NCORES):
        b, g = divmod(c, CPB)
        cs = slice(g * COLS, (g + 1) * COLS)
        in_maps.append({
            "qT": qT[b],
            "kT": kT[b],
            "vT": vT[b],
            "wq": np.ascontiguousarray(Wq[:, cs]).astype(bf),
            "wk": np.ascontiguousarray(Wk[:, cs]).astype(bf),
            "wv": np.ascontiguousarray(Wv[:, cs]).astype(bf),
            "wo": np.ascontiguousarray(Wo[cs, :]).astype(bf),
            "bqr": np.ascontiguousarray(bq[cs].reshape(NG, 128).T),
            "bkr": np.ascontiguousarray(bk[cs].reshape(NG, 128).T),
            "bvr": np.ascontiguousarray(
                np.broadcast_to(bv[cs].reshape(1, COLS), (128, COLS))),
            "bor": np.ascontiguousarray(np.broadcast_to(
                (bo if g == 0 else np.zeros_like(bo)).reshape(1, D), (128, D))),
        })
    return in_maps


def combine_outputs(results):
    out = np.zeros((B, S, D), np.float32)
    for c in range(NCORES):
        out[c // CPB] += np.asarray(results[c]["out"], np.float32)
    return out


def kernel(q, k, v, Wq, Wk, Wv, Wo, bq, bk, bv, bo):
    from concourse.bass_utils import run_bass_kernel_spmd

    nc = _get_program()
    in_maps = make_in_maps(q, k, v, Wq, Wk, Wv, Wo, bq, bk, bv, bo)
    res = run_bass_kernel_spmd(nc, in_maps, list(range(NCORES)))
    return combine_outputs(res.results)
